# revision 33
# baseline (speedup 1.0000x reference)
"""Distributed Trainium2 Bass kernel for gnn_message_passing (8 NeuronCores).

Strategy (atom/target sharding, graph-parallel):
  - Atoms are partitioned into 8 contiguous target shards (3750 atoms each).
  - Each edge type's edges are sorted by target and assigned to the target's
    owner core.  Each core processes its edges in fixed-width target windows;
    per window it gathers source rows (dma_gather), expands each edge's
    feature outer-product Z = F' (x) xn on DVE/ACT, and uses the TensorEngine
    to both segment-sum (one-hot scatter matmul into PSUM, G^T orientation)
    and apply the edge-network weight contraction (out = G @ Wr).
  - After the bond and angle message sub-steps and after each GRU step, the
    cores exchange their updated shard rows with an AllGather so the next
    sub-step's gathers (whose sources are random atoms) see the full table.
  - All edge schedules / one-hot scatter matrices / wrapped gather indices
    are precomputed on the host from the actual input indices and shipped as
    per-core tensors; the compiled program is identical across cores.
"""

import os
import sys

sys.path.insert(0, "/opt/trn_rl_repo")

import numpy as np
import ml_dtypes

import concourse.bass as bass
import concourse.mybir as mybir
import concourse.tile as tile
from concourse import library_config
from concourse.library_overlay import lower_extended_insts
from concourse.tile_rust import add_dep_helper
from concourse.masks import make_identity
from concourse.bass_utils import run_bass_kernel_spmd

# ---------------------------------------------------------------- tile patch
# This walrus build accepts at most ONE sync wait per instruction; spread
# extra waits across same-engine nops placed right before the instruction.
from concourse.tile import TileContext
from concourse.vector_clock import ScopedClock

_orig_drain_and_barrier = TileContext._drain_and_barrier


def _patched_drain_and_barrier(self, tick_clock, wait_clock):
    nc = self.nc
    probe = nc.sync.nop(nofuse=True)
    wait_clock.add_sem_waits(probe.ins, ScopedClock({None: tick_clock.global_clock}))
    si = probe.ins.sync_info
    waits = list(si.on_wait) if si is not None and si.on_wait else []
    if si is not None:
        si.on_wait = waits[:1]
    for w in waits[1:]:
        nop = nc.sync.nop(nofuse=True)
        nop.ins.sync_info = mybir.SyncInfo(on_wait=[w], on_update=[])
    nc.sync.drain()
    nc.all_engine_barrier()
    popped = nc._tile_sem_poison_stack.pop()
    assert popped is self._sem_poison
    nc.clear_and_free_semaphores(list(self.sems.allocated().values()))
    nc.all_engine_barrier()


TileContext._drain_and_barrier = _patched_drain_and_barrier


def _split_multi_waits(nc, max_waits=1):
    n = 0
    for f in nc.m.functions:
        for bb in f.blocks:
            out = []
            for inst in bb.instructions:
                si = inst.sync_info
                if si is not None and si.on_wait and len(si.on_wait) > max_waits:
                    waits = list(si.on_wait)
                    for w in waits[:-max_waits]:
                        nop = mybir.InstNoOp(
                            name=f"wsplit-{nc.next_id()}", ins=[], outs=[],
                            engine=inst.engine)
                        nop.sync_info = mybir.SyncInfo(on_wait=[w], on_update=[])
                        try:
                            nc.register_instruction(nop, overwrite=True)
                        except Exception:
                            pass
                        out.append(nop)
                        n += 1
                    si.on_wait = waits[-max_waits:]
                out.append(inst)
            bb.instructions = out
    return n


# ------------------------------------------------------------------- config
NCORES = 8
STEPS = int(os.environ.get("KSTEPS", "4"))
D = 64
BF16 = mybir.dt.float16  # fp16: 10-bit mantissa, full PE rate
F32 = mybir.dt.float32
I16 = mybir.dt.int16

# per-edge-type max window width (targets per window); windows are built
# greedily so each holds <=128 edges on every core.  Wmax bounds PSUM use:
# ceil(cb / (512 // Wmax)) <= 2 banks for the G^T tile.
TYPE_CFG = {
    "bond": dict(Wmax=102),      # cb=9  -> bpb>=5 (2 PSUM banks)
    "angle": dict(Wmax=128),     # width is also the out-mm partition count
    "dihedral": dict(Wmax=128),  # cb=7  -> bpb>=4
}

_last_results = {}  # test.py introspection


# --------------------------------------------------------------- host prep
def _wrap_idx(idx):
    """dma_gather index layout: [128, n/16]; partition p in [0,16) holds
    idx[p::16]; replicated across the 8 groups of 16 partitions."""
    n = len(idx)
    assert n % 128 == 0
    w = np.zeros((128, n // 16), dtype=np.int16)
    for p in range(16):
        w[p, :] = idx[p::16]
    w[16:, :] = np.tile(w[:16, :], (7, 1))
    return w


def _ag_remap(n_atoms, shard):
    """Row map for the (lo-block | hi-block) split-allgather table layout:
    atom a -> table row; halves of each core's shard gathered separately."""
    a = np.arange(n_atoms, dtype=np.int64)
    c, r = a // shard, a % shard
    half = shard // 2
    return np.where(r < half, c * half + r,
                    NCORES * half + c * half + (r - half)).astype(np.int64)


def _prep_type(pair_idx, feat, n_atoms, shard, Wmax, remap):
    """Build per-core schedules + packed tensors for one edge type.

    Windows are variable-width, chosen greedily so the max-over-cores edge
    count per window is <=128 (one chunk per window on every core).
    Returns (sched, per_core, nch, util); sched = [(row0, width)] identical
    across cores.
    """
    E, f = feat.shape
    tgt = pair_idx[:, 0].astype(np.int64)
    src = remap[pair_idx[:, 1].astype(np.int64)]
    core = tgt // shard
    tloc = tgt % shard

    # per-target per-core counts -> prefix sums [NCORES, shard+1]
    cnt = np.zeros((NCORES, shard), dtype=np.int64)
    for c in range(NCORES):
        cnt[c] = np.bincount(tloc[core == c], minlength=shard)
    pre = np.concatenate([np.zeros((NCORES, 1), np.int64),
                          np.cumsum(cnt, axis=1)], axis=1)

    sched = []  # (row0, width)
    lo = 0
    while lo < shard:
        hi = lo + 1
        while (hi < shard and hi - lo < Wmax
               and int((pre[:, hi + 1] - pre[:, lo]).max()) <= 128):
            hi += 1
        sched.append((lo, hi - lo))
        lo = hi
    nch = len(sched)
    row0s = np.array([r for (r, _) in sched], dtype=np.int64)
    widths = np.array([w for (_, w) in sched], dtype=np.int64)
    Weff = int(widths.max())

    per_core = []
    for c in range(NCORES):
        sel = core == c
        t_c = tloc[sel]
        s_c = src[sel]
        f_c = feat[sel]
        w_c = np.searchsorted(row0s, t_c, side="right") - 1
        idx_all = np.zeros(nch * 128, dtype=np.int16)
        F_all = np.zeros((128, nch, f + 1), dtype=np.float32)
        S_all = np.zeros((128, nch, Weff), dtype=np.float32)
        eorder = np.argsort(w_c, kind="stable")
        t_c, s_c, f_c, w_c = t_c[eorder], s_c[eorder], f_c[eorder], w_c[eorder]
        b = np.concatenate([[0], np.cumsum(np.bincount(w_c, minlength=nch))])
        for w in range(nch):
            lo, hi = b[w], b[w + 1]
            n_e = hi - lo
            assert n_e <= 128, (w, n_e)
            ps = np.arange(n_e)
            idx_all[w * 128 + ps] = s_c[lo:hi].astype(np.int16)
            F_all[ps, w, :f] = f_c[lo:hi]
            F_all[ps, w, f] = 1.0
            S_all[ps, w, t_c[lo:hi] - row0s[w]] = 1.0
        per_core.append(dict(
            idx=_wrap_idx(idx_all),
            F=F_all,
            S=S_all.astype(np.float16),
        ))
    util = E / (nch * 128 * NCORES)
    return sched, per_core, nch, util, Weff


def _pack_wr(Wt, bt, f):
    """Wr[k*64+j, i] = Wt[k, i*64+j]; bias block at k=f; pad to CB*128 rows."""
    cb = ((f + 1) * D + 127) // 128
    Wr = np.zeros((cb * 128, D), dtype=np.float32)
    Kr = Wt.reshape(f, D, D)
    for k in range(f):
        Wr[k * D:(k + 1) * D, :] = Kr[k].T
    Wr[f * D:(f + 1) * D, :] = bt.reshape(D, D).T
    # SBUF layout [128, cb, 64]
    return np.ascontiguousarray(
        Wr.reshape(cb, 128, D).transpose(1, 0, 2)).astype(np.float16)


# ------------------------------------------------------------ kernel build
def _build(n_atoms, scheds, nchs, fdims, weffs, trace_label=""):
    shard = n_atoms // NCORES
    nc = bass.Bass(num_devices=NCORES)
    types = ["bond", "angle", "dihedral"]
    cbs = {t: ((fdims[t] + 1) * D + 127) // 128 for t in types}

    # ---- I/O tensors
    x0_full = nc.dram_tensor("x0_full", [n_atoms, D], F32, kind="ExternalInput")
    h0_shard = nc.dram_tensor("h0_shard", [shard, D], F32, kind="ExternalInput")
    ins = {}
    for t in types:
        f = fdims[t]
        W = weffs[t]
        ins[t] = dict(
            idx=nc.dram_tensor(f"{t}_idx", [128, nchs[t] * 8], I16, kind="ExternalInput"),
            F=nc.dram_tensor(f"{t}_F", [128, nchs[t], f + 1], F32, kind="ExternalInput"),
            S=nc.dram_tensor(f"{t}_S", [128, nchs[t], W], BF16, kind="ExternalInput"),
            Wr=nc.dram_tensor(f"{t}_Wr", [128, cbs[t], D], BF16, kind="ExternalInput"),
        )
    wiT = nc.dram_tensor("wiT", [65, 3 * D], BF16, kind="ExternalInput")
    whT = nc.dram_tensor("whT", [65, 3 * D], BF16, kind="ExternalInput")
    out_shard = nc.dram_tensor("out_shard", [shard, D], F32, kind="ExternalOutput")

    # ---- internal DRAM: shard buffers + allgathered tables per step
    a_shard = {}   # message outputs (shard rows)
    h_shard = {}
    tables = {}    # allgathered full tables
    for s in range(STEPS):
        for t in ("a1", "a2"):
            tables[(s, t)] = nc.dram_tensor(
                f"tab_{t}_{s}", [n_atoms, D], F32, addr_space="Shared")
        if s < STEPS - 1:
            tables[(s, "h")] = nc.dram_tensor(
                f"tab_h_{s}", [n_atoms, D], F32, addr_space="Shared")

    with tile.TileContext(nc) as tc:
        with (
            tc.tile_pool(name="const", bufs=1) as cpool,
            tc.tile_pool(name="work", bufs=3) as pool,
            tc.tile_pool(name="dram", bufs=1, space="DRAM") as dpool,
            tc.tile_pool(name="psum", bufs=2, space="PSUM") as psum,
        ):
            nc.gpsimd.load_library(library_config.mlp)

            _reg_cache = {}

            def idx_reg(v):
                if v not in _reg_cache:
                    _reg_cache[v] = nc.gpsimd.to_reg(v)
                return _reg_cache[v]



            # ---- persistent SBUF constants
            ct = {}
            for t in types:
                f = fdims[t]
                W = weffs[t]
                it = ins[t]
                idx_t = cpool.tile([128, nchs[t] * 8], I16, name=f"idx_{t}")
                nc.sync.dma_start(out=idx_t[:], in_=it["idx"][:])
                F_t = cpool.tile([128, nchs[t], fdims[t] + 1], F32, name=f"F_{t}")
                nc.sync.dma_start(out=F_t[:], in_=it["F"][:])
                S_t = cpool.tile([128, nchs[t], W], BF16, name=f"S_{t}")
                nc.sync.dma_start(out=S_t[:], in_=it["S"][:])
                Wr_t = cpool.tile([128, cbs[t], D], BF16, name=f"Wr_{t}")
                nc.sync.dma_start(out=Wr_t[:], in_=it["Wr"][:])
                ct[t] = dict(idx=idx_t, F=F_t, S=S_t, Wr=Wr_t)
            wiT_t = cpool.tile([65, 3 * D], BF16, name="wiT_t")
            nc.sync.dma_start(out=wiT_t[:], in_=wiT[:])
            whT_t = cpool.tile([65, 3 * D], BF16, name="whT_t")
            nc.sync.dma_start(out=whT_t[:], in_=whT[:])
            ident = cpool.tile([128, 128], F32, name="ident")
            make_identity(nc, ident[:])

            # shard outputs as DRAM pool tiles (dep-tracked)
            for s in range(STEPS):
                for t in ("a1", "a2", "a3"):
                    a_shard[(s, t)] = dpool.tile([shard, D], F32, name=f"a_{t}_{s}")
                h_shard[s] = dpool.tile([shard, D], F32, name=f"h_{s}")

            # persistent per-type gathered-source buffers (whole substep)
            xnbufs = {}
            for t in types:
                xnbufs[t] = cpool.tile([128, nchs[t], D], F32, name=f"xnb_{t}")

            GRP = 7  # chunks per gather call (896 idx = 56 descs/engine)

            def run_gathers(t, src_table_ap, table_deps):
                """Immediate-fire gathers for a substep, GRP chunks per call."""
                c = ct[t]
                sched = scheds[t]
                xnb = xnbufs[t]
                ch_base = 0
                for i in range(0, len(sched), GRP):
                    g_ch = min(GRP, len(sched) - i)
                    gather = nc.gpsimd.dma_gather(
                        out_ap=xnb[:, ch_base:ch_base + g_ch, :],
                        in_ap=src_table_ap,
                        idxs_ap=c["idx"][:, ch_base * 8:(ch_base + g_ch) * 8],
                        num_idxs=g_ch * 128,
                        num_idxs_reg=idx_reg(g_ch * 128),
                        elem_size=D,
                    )
                    for dep in table_deps:
                        add_dep_helper(gather.ins, dep.ins,
                                       reason="gather waits on allgather")
                    ch_base += g_ch

            def message_substep(step, t, dst_shard):
                """Run one edge-type message pass (sources gathered into
                xnbufs[t] by run_gathers)."""
                f = fdims[t]
                fb = f + 1
                cb = cbs[t]
                sched = scheds[t]
                c = ct[t]
                xnb = xnbufs[t]
                if True:
                    for ci, (row0, width) in enumerate(sched):
                        w = ci
                        bpb = 512 // width  # matmul out must stay in one bank
                        gt = psum.tile([128, 1024], F32, tag="gt",
                                       name=f"gt_{t}_{step}_{w}")

                        def gcol(b):
                            return (b // bpb) * 512 + (b % bpb) * width

                        # z[e, (k,j)] = F[e,k] * xn[e,j] in ONE DVE op
                        z = pool.tile([128, fb * D], BF16, tag="z",
                                      name=f"z_{t}_{step}_{w}")
                        F_bc = (c["F"][:, ci, :].unsqueeze(2)
                                .broadcast_to([128, fb, D]))
                        x_bc = (xnb[:, ci, :].unsqueeze(1)
                                .broadcast_to([128, fb, D]))
                        nc.vector.tensor_mul(
                            z[:].rearrange("p (a b) -> p a b", a=fb),
                            F_bc, x_bc)
                        for b in range(cb):
                            cw = min(128, fb * D - b * 128)
                            nc.tensor.matmul(
                                gt[:cw, gcol(b):gcol(b) + width],
                                lhsT=z[:, b * 128:b * 128 + cw],
                                rhs=c["S"][:, ci, :width],
                                start=True, stop=True)
                        # drain G^T to SBUF fp16, one op per PSUM bank
                        gtsb = pool.tile([128, cb, width], BF16, tag="gtsb",
                                         name=f"gtsb_{t}_{step}_{w}", bufs=2)
                        b0 = 0
                        while b0 < cb:
                            nblk = min(bpb, cb - b0)
                            nc.scalar.activation(
                                gtsb[:, b0:b0 + nblk, :],
                                gt[:, b0 // bpb * 512:b0 // bpb * 512 + nblk * width],
                                mybir.ActivationFunctionType.Copy)
                            b0 += nblk
                        # out-mm: out[tl, i] = sum_b G^T_b[:, tl].T @ Wr_b
                        ops = psum.tile([128, D], F32, tag="ops",
                                        name=f"ops_{t}_{step}_{w}")
                        pmm = None
                        for b in range(cb):
                            cw = min(128, fb * D - b * 128)
                            mm = nc.tensor.matmul(
                                ops[:width, :], lhsT=gtsb[:cw, b, :width],
                                rhs=c["Wr"][:cw, b, :],
                                start=(b == 0), stop=(b == cb - 1))
                            if pmm is not None:
                                add_dep_helper(mm.ins, pmm.ins, reason="psum accum order")
                            pmm = mm
                        osb = pool.tile([128, D], F32, tag="osb",
                                        name=f"osb_{t}_{step}_{w}", bufs=2)
                        nc.scalar.activation(osb[:width, :], ops[:width, :],
                                             mybir.ActivationFunctionType.Copy)
                        nc.sync.dma_start(
                            out=dst_shard[row0:row0 + width, :],
                            in_=osb[:width, :])

            def allgather(shard_tile, full_tensor):
                """Two half-shard AllGathers: the lo half can fire while the
                windows covering the hi half are still computing.  Gather
                indices are host-remapped to the (lo-block | hi-block)
                table layout this produces."""
                half = shard // 2
                ccs = []
                for (a, b, o) in ((0, half, 0),
                                  (half, shard, NCORES * half)):
                    cc = nc.gpsimd.collective_compute(
                        "AllGather",
                        mybir.AluOpType.bypass,
                        replica_groups=[list(range(NCORES))],
                        ins=[shard_tile[a:b]],
                        outs=[full_tensor[o:o + NCORES * (b - a)]],
                    )
                    ccs.append(cc)
                return ccs

            def gru_step(step, a3, hprev_ap, dst_h, dst_out):
                nwin = (shard + 127) // 128
                for w in range(nwin):
                    rows = min(128, shard - w * 128)
                    sl = slice(w * 128, w * 128 + rows)
                    x_sb = pool.tile([128, D], F32, tag="gx", name=f"gx_{step}_{w}", bufs=2)
                    nc.sync.dma_start(out=x_sb[:rows, :], in_=a3[sl, :])
                    h_sb = pool.tile([128, D], F32, tag="gh", name=f"gh_{step}_{w}", bufs=2)
                    nc.sync.dma_start(out=h_sb[:rows, :], in_=hprev_ap[sl, :])
                    # packed GRU PSUM bank: rz[0:128] (accumulating -> bank
                    # col 0), inp[128:192], hnp[192:256], xt[256:384], ht[384:512]
                    gp = psum.tile([128, 512], F32, tag="gp", name=f"gp_{step}_{w}")
                    nc.tensor.transpose(out=gp[:D, 256:384], in_=x_sb[:, :], identity=ident[:])
                    nc.tensor.transpose(out=gp[:D, 384:512], in_=h_sb[:, :], identity=ident[:])
                    xa = pool.tile([65, 128], BF16, tag="xa", name=f"xa_{step}_{w}", bufs=2)
                    nc.scalar.activation(xa[:D, :], gp[:D, 256:384],
                                         mybir.ActivationFunctionType.Copy)
                    nc.vector.memset(xa[D:65, :], 1.0)
                    ha = pool.tile([65, 128], BF16, tag="ha", name=f"ha_{step}_{w}", bufs=2)
                    nc.vector.tensor_copy(ha[:D, :], gp[:D, 384:512])
                    nc.vector.memset(ha[D:65, :], 1.0)
                    # gate matmuls
                    mm1 = nc.tensor.matmul(gp[:, 0:128], lhsT=xa[:, :], rhs=wiT_t[:, 0:2 * D],
                                     start=True, stop=False)
                    mm2 = nc.tensor.matmul(gp[:, 0:128], lhsT=ha[:, :], rhs=whT_t[:, 0:2 * D],
                                     start=False, stop=True)
                    add_dep_helper(mm2.ins, mm1.ins, reason="psum accum order")
                    nc.tensor.matmul(gp[:, 128:192], lhsT=xa[:, :], rhs=wiT_t[:, 2 * D:],
                                     start=True, stop=True)
                    nc.tensor.matmul(gp[:, 192:256], lhsT=ha[:, :], rhs=whT_t[:, 2 * D:],
                                     start=True, stop=True)
                    # elementwise
                    rzs = pool.tile([128, 2 * D], F32, tag="rzs", name=f"rzs_{step}_{w}", bufs=2)
                    nc.scalar.activation(rzs[:, :], gp[:, 0:128],
                                         mybir.ActivationFunctionType.Sigmoid)
                    t1 = pool.tile([128, D], F32, tag="t1", name=f"t1_{step}_{w}", bufs=2)
                    nc.vector.tensor_mul(t1[:, :], rzs[:, :D], gp[:, 192:256])
                    nc.vector.tensor_add(t1[:, :], t1[:, :], gp[:, 128:192])
                    nn_ = pool.tile([128, D], F32, tag="nn", name=f"nn_{step}_{w}", bufs=2)
                    nc.scalar.activation(nn_[:, :], t1[:, :],
                                         mybir.ActivationFunctionType.Tanh)
                    # h' = n + z*(h - n)
                    t2 = pool.tile([128, D], F32, tag="t2", name=f"t2_{step}_{w}", bufs=2)
                    nc.vector.tensor_sub(t2[:, :], h_sb[:, :], nn_[:, :])
                    nc.vector.tensor_mul(t2[:, :], t2[:, :], rzs[:, D:])
                    hout = pool.tile([128, D], F32, tag="hout", name=f"hout_{step}_{w}", bufs=2)
                    nc.vector.tensor_add(hout[:, :], nn_[:, :], t2[:, :])
                    if dst_h is not None:
                        nc.sync.dma_start(out=dst_h[sl, :], in_=hout[:rows, :])
                    if dst_out is not None:
                        nc.sync.dma_start(out=dst_out[sl, :], in_=hout[:rows, :])

            # ---------------- main program
            if os.environ.get("KONLY") == "bond":
                run_gathers("bond", x0_full[:], [])
                message_substep(0, "bond", a_shard[(0, "a1")])
                nc.sync.dma_start(out=out_shard[:], in_=a_shard[(0, "a1")][:])
                _steps = 0
            else:
                _steps = STEPS
            h_cc = []
            for s in range(_steps):
                htab_ap = x0_full[:] if s == 0 else tables[(s - 1, "h")][:]
                hprev_ap = h0_shard[:] if s == 0 else h_shard[s - 1][:]
                # ---- bond
                run_gathers("bond", htab_ap, h_cc)
                message_substep(s, "bond", a_shard[(s, "a1")])
                cc1 = allgather(a_shard[(s, "a1")], tables[(s, "a1")])
                # ---- angle
                run_gathers("angle", tables[(s, "a1")][:], cc1)
                message_substep(s, "angle", a_shard[(s, "a2")])
                cc2 = allgather(a_shard[(s, "a2")], tables[(s, "a2")])
                # ---- dihedral
                run_gathers("dihedral", tables[(s, "a2")][:], cc2)
                message_substep(s, "dihedral", a_shard[(s, "a3")])
                gru_step(s, a_shard[(s, "a3")][:], hprev_ap,
                         h_shard[s] if s < STEPS - 1 else None,
                         out_shard[:] if s == STEPS - 1 else None)
                if s < STEPS - 1:
                    h_cc = allgather(h_shard[s], tables[(s, "h")])

    lower_extended_insts(nc)
    _split_multi_waits(nc)
    return nc


# ------------------------------------------------------------------ public
def kernel(**inputs):
    af = np.asarray(inputs["atom_features"], dtype=np.float32)
    n_atoms = af.shape[0]
    shard = n_atoms // NCORES

    scheds, nchs, fdims, per_core, weffs = {}, {}, {}, {}, {}
    spec = [
        ("bond", "bond_features", "pair_indices", "W_edge", "b_edge"),
        ("angle", "bond_angle_features", "bond_angle_pair_indices", "W_angle", "b_angle"),
        ("dihedral", "dihedral_angle_features", "dihedral_angle_pair_indices",
         "W_dihedral", "b_dihedral"),
    ]
    wrs = {}
    remap = _ag_remap(n_atoms, shard)
    for t, fk, ik, wk, bk in spec:
        feat = np.asarray(inputs[fk], dtype=np.float32)
        pi = np.asarray(inputs[ik])
        fdims[t] = feat.shape[1]
        sched, pc, nch, util, weff = _prep_type(
            pi, feat, n_atoms, shard, TYPE_CFG[t]["Wmax"], remap)
        scheds[t], per_core[t], nchs[t], weffs[t] = sched, pc, nch, weff
        wrs[t] = _pack_wr(np.asarray(inputs[wk], np.float32),
                          np.asarray(inputs[bk], np.float32), fdims[t])
        if os.environ.get("KVERBOSE"):
            print(f"[{t}] windows={len(sched)} chunks={nch} util={util:.2f} "
                  f"weff={weff}")

    wi = np.asarray(inputs["gru_wi"], np.float32)   # [3h, h]
    wh = np.asarray(inputs["gru_wh"], np.float32)
    bi = np.asarray(inputs["gru_bi"], np.float32)
    bh = np.asarray(inputs["gru_bh"], np.float32)
    wiT = np.concatenate([wi.T, bi[None, :]], 0).astype(np.float16)  # [65, 192]
    whT = np.concatenate([wh.T, bh[None, :]], 0).astype(np.float16)

    nc = _build(n_atoms, scheds, nchs, fdims, weffs)

    x0_remapped = np.empty_like(af)
    x0_remapped[remap] = af

    in_maps = []
    for c in range(NCORES):
        m = dict(
            x0_full=x0_remapped,
            h0_shard=np.ascontiguousarray(af[c * shard:(c + 1) * shard]),
            wiT=wiT, whT=whT,
        )
        for t in ("bond", "angle", "dihedral"):
            pc = per_core[t][c]
            m[f"{t}_idx"] = pc["idx"]
            m[f"{t}_F"] = pc["F"]
            m[f"{t}_S"] = pc["S"]
            m[f"{t}_Wr"] = wrs[t]
        in_maps.append(m)

    if os.environ.get("KBUILD_ONLY"):
        _last_results["nc"] = nc
        _last_results["in_maps"] = in_maps
        return np.zeros((n_atoms, D), dtype=np.float32)
    if os.environ.get("KTIME"):
        results = _run_timed(nc, in_maps)
    else:
        res = run_bass_kernel_spmd(nc, in_maps, list(range(NCORES)))
        _last_results["exec_time_ns"] = res.exec_time_ns
        results = res.results

    out = np.zeros((n_atoms, D), dtype=np.float32)
    for c in range(NCORES):
        out[c * shard:(c + 1) * shard] = results[c]["out_shard"]
    return out


def _run_timed(nc, in_maps, n_iters=3):
    """Replicates bass2jax.run_bass_via_pjrt but with device-resident inputs
    and repeated execution so the min wall time approximates HW exec time."""
    import time
    import jax
    from jax.sharding import Mesh, PartitionSpec
    from jax.experimental.shard_map import shard_map
    from concourse import bass2jax
    from concourse.bass2jax import _bass_exec_p, partition_id_tensor

    bass2jax.install_neuronx_cc_hook()
    n_cores = NCORES
    partition_name = nc.partition_id_tensor.name if nc.partition_id_tensor else None
    in_names, out_names, out_avals, zero_outs = [], [], [], []
    for alloc in nc.m.functions[0].allocations:
        if not isinstance(alloc, mybir.MemoryLocationSet):
            continue
        name = alloc.memorylocations[0].name
        if alloc.kind == "ExternalInput":
            if name != partition_name:
                in_names.append(name)
        elif alloc.kind == "ExternalOutput":
            out_names.append(name)
            shape = tuple(alloc.tensor_shape)
            dtype = mybir.dt.np(alloc.dtype)
            out_avals.append(jax.core.ShapedArray(shape, dtype))
            zero_outs.append(np.zeros(shape, dtype))
    n_params = len(in_names)
    all_in_names = list(in_names) + list(out_names)
    if partition_name is not None:
        all_in_names.append(partition_name)

    def _body(*args):
        operands = list(args)
        if partition_name is not None:
            operands.append(partition_id_tensor())
        outs = _bass_exec_p.bind(
            *operands,
            out_avals=tuple(out_avals),
            in_names=tuple(all_in_names),
            out_names=tuple(out_names),
            lowering_input_output_aliases=(),
            sim_require_finite=True,
            sim_require_nnan=True,
            nc=nc,
        )
        return tuple(outs)

    devices = jax.devices()[:n_cores]
    mesh = Mesh(np.asarray(devices), ("core",))
    spec = PartitionSpec("core")
    in_specs = (spec,) * (n_params + len(out_names))
    sharded = jax.jit(shard_map(_body, mesh=mesh, in_specs=in_specs,
                                out_specs=(spec,) * len(out_names),
                                check_rep=False), keep_unused=True)
    concat_in = [np.concatenate([np.asarray(in_maps[c][nm]) for c in range(n_cores)], 0)
                 for nm in in_names]
    concat_zeros = [np.zeros((n_cores * z.shape[0], *z.shape[1:]), z.dtype)
                    for z in zero_outs]
    sh = jax.sharding.NamedSharding(mesh, spec)
    dev_in = [jax.device_put(a, sh) for a in concat_in + concat_zeros]
    out = sharded(*dev_in)
    jax.block_until_ready(out)
    times = []
    for _ in range(n_iters):
        t0 = time.perf_counter()
        out = sharded(*dev_in)
        jax.block_until_ready(out)
        times.append(time.perf_counter() - t0)
    _last_results["exec_time_ns"] = int(min(times) * 1e9)
    _last_results["times"] = times
    return [
        {nm: np.asarray(out[i]).reshape(n_cores, *out_avals[i].shape)[c]
         for i, nm in enumerate(out_names)}
        for c in range(n_cores)
    ]

